# revision 39
# baseline (speedup 1.0000x reference)
"""Bhattacharyya coefficient kernel for Trainium2 (8 NeuronCores, SPMD).

out[n,0,i,j] = (1/k^2) * sum_{c,p,q} w[c] * sqrt(x[n,c,i+p,j+q] * z[n,c,p,q])

Data-parallel over batch, 2 samples per core.  Per sample:
  1. ACT: sx = sqrt(x) (bf16), szw = w/k^2 * sqrt(z) (bf16); sqrt(x*z)
     factorizes so the whole unfold collapses into a cross-correlation.
  2. TensorE: plane[t, y] = sum_c szw[c, t] * sx[c, y] for the 64 taps
     t = 8p+q and all 63*63 pixels y (K=256 as two accumulating
     128-chunks, 512-column PSUM blocks).
  3. DVE evicts PSUM into fp8-e4m3 plane pieces sized exactly like the
     DRAM scratch tensors (one dump DMA each, exact dependencies).
     fp8 halves scratch traffic; the tap-sum averages 64 independent
     quantization errors so absmax rel err stays ~1.1e-2 (< 2e-2 gate).
  4. Dump to DRAM scratch (Sync ring: FIFO behind the x stream, i.e.
     the transfers fill the ramp-down of the stream) and gather back
     tap-aligned with a flat-DRAM diagonal AP
       A[t, u] = plane[t, u + 63*(t>>3) + (t&7)]     (SWDGE ring),
     which turns the tap-sum into a pure partition reduction.
  5. Strictly after both samples' stage-1 (engines run in near-emission
     order; interleaving that mismatches readiness serializes the
     kernel): per chunk a K=64 ones-matmul, column-tiled so chunk ch
     accumulates on PSUM partition 32ch.  Chunk 3 (which carries the
     last x block) gets its own PSUM and SBUF tiles so neither its
     matmul nor its eviction waits on tile-granular hazards against the
     chunk 0-2 path, and its eviction runs on the idle ACT engine.
     Chunks 0+1 and chunk 2 evict into separate obuf tiles so each out
     DMA's tile-granular RAW covers only its own eviction - rows 0-31
     ship as soon as chunks 0/1 reduce.

The x stream owns the Sync HWDGE ring (12 piece loads issued
back-to-back after the tiny z/w loads) and runs at the ~358 GB/s HBM
limit for ~23us; everything else hides behind it except sample 1's
last-block tail.  Sample-1 gathers ride the drained Sync ring while sample-0
uses SWDGE, and keep-warm matmuls bridge the post-stage-1 PE gap so
HAM holds full clock into the tail.  Measured 47.5us (window-best) vs
the 55.5us baseline; run-to-run HW variance is +-3us.
"""

import numpy as np

import concourse.bacc as bacc
import concourse.bass as bass
import concourse.mybir as mybir
from concourse import tile
from concourse.bass_utils import run_bass_kernel_spmd

N, C, KS, MS = 16, 256, 8, 63
MO = MS - KS + 1            # 56
F = MS * MS                 # 3969
NCORES = 8
SPC = N // NCORES           # samples per core
BLK = 512
W = (MO - 1) * MS + MO      # 3521
SH = 448
AF = mybir.ActivationFunctionType
f32 = mybir.dt.float32
bf16 = mybir.dt.bfloat16
fp8 = mybir.dt.float8e4

PIECES = [(0, 4), (4, 3), (7, 1)]
GCH = [(0, 1008), (1008, 2016), (2016, 3024), (3024, W)]
SCR = [(0, 2016 + SH), (2016, 3024 + SH), (3024, F)]
CHUNK_SC = [0, 0, 1, 2]
OUT_ROWS = {1: (0, 32), 3: (32, MO)}

_CACHE = {}


def _build():
    nc = bacc.Bacc("TRN2", target_bir_lowering=False, debug=False)
    z_in = nc.declare_dram_parameter("z", [SPC, C, KS, KS], f32, isOutput=False)
    x_in = nc.declare_dram_parameter("x", [SPC, C, MS, MS], f32, isOutput=False)
    w_in = nc.declare_dram_parameter("w", [C], f32, isOutput=False)
    out = nc.declare_dram_parameter("out", [SPC, 1, MO, MO], f32, isOutput=True)

    scs = [
        [nc.dram_tensor(f"sc{ci}_{s}", [64, c1 - c0], fp8)
         for ci, (c0, c1) in enumerate(SCR)]
        for s in range(SPC)
    ]

    xflat = x_in.rearrange("s (k c) h w -> s k c (h w)", c=128)

    with tile.TileContext(nc) as tc:
        with (
            tc.tile_pool(name="xstage", bufs=12) as xstage,
            tc.tile_pool(name="sxq", bufs=5) as sxq,
            tc.tile_pool(name="zpool", bufs=2) as zpool,
            tc.tile_pool(name="plane", bufs=2) as plane,
            tc.tile_pool(name="gath", bufs=8) as gath,
            tc.tile_pool(name="opool", bufs=1) as opool,
            tc.tile_pool(name="psum", bufs=4, space="PSUM") as psum,
            tc.tile_pool(name="psum2", bufs=1, space="PSUM") as psum2,
            tc.tile_pool(name="psum3", bufs=2, space="PSUM") as psum3,
        ):
            xst = {}

            def load_piece(s, k, pi):
                b0, nbk = PIECES[pi]
                lo = b0 * BLK
                ln = min(nbk * BLK, F - lo)
                t = xstage.tile([128, 4 * BLK], f32, tag="xst",
                                name=f"xst{s}{k}{pi}")
                nc.sync.dma_start(t[:, :ln], xflat[s, k, :, lo : lo + ln])
                xst[(s, k, pi)] = t

            load_piece(0, 0, 0)
            wt = zpool.tile([128, 2], f32, name="wt")
            nc.sync.dma_start(wt[:], w_in.rearrange("(k c) -> c k", c=128))
            zts = []
            for s in range(SPC):
                zt = zpool.tile([128, 2, KS * KS], f32, tag="zt", name=f"zt{s}")
                nc.sync.dma_start(
                    zt[:], z_in[s].rearrange("(k c) p q -> c k (p q)", c=128)
                )
                zts.append(zt)
            for s in range(SPC):
                for pi in range(len(PIECES)):
                    for k in range(2):
                        if (s, k, pi) != (0, 0, 0):
                            load_piece(s, k, pi)

            ones = opool.tile([64, 1], fp8, name="ones")
            nc.gpsimd.memset(ones[:], 1.0)
            w64 = zpool.tile([128, 2], f32, name="w64")
            nc.vector.tensor_scalar_mul(w64[:], wt[:], 1.0 / (KS * KS))

            obufs, obufBs, obuf3s, psum2s, psum3s, ats = ({}, {}, {}, {},
                                                         {}, {})
            deferred_gathers = []
            for s in range(SPC):
                obuf = opool.tile([128, 1024], f32, tag=f"ob{s}",
                                  name=f"obuf{s}")
                obufs[s] = obuf
                obuf3s[s] = opool.tile([128, 512], f32, tag=f"ob3{s}",
                                       name=f"obuf3_{s}")
                obufBs[s] = opool.tile([128, 1024], f32, tag=f"obB{s}",
                                       name=f"obufB_{s}")
                psum2s[s] = psum2.tile([128, 2 * BLK], f32, tag="ps2",
                                       name=f"ps2_{s}")
                psum3s[s] = psum3.tile([128, BLK], f32, tag="ps3",
                                       name=f"ps3_{s}")
                zsq = zpool.tile([128, 2, KS * KS], f32, tag="zsq", name=f"zsq{s}")
                szw = zpool.tile([128, 2, KS * KS], bf16, tag="szw", name=f"szw{s}")
                for kk in range(2):
                    nc.scalar.activation(zsq[:, kk, :], zts[s][:, kk, :], AF.Sqrt)
                    nc.vector.tensor_scalar_mul(
                        szw[:, kk, :], zsq[:, kk, :], w64[:, kk : kk + 1]
                    )

                pls = [
                    plane.tile([64, c1 - c0], fp8, tag=f"pl{ci}",
                               name=f"pl{s}_{ci}")
                    for ci, (c0, c1) in enumerate(SCR)
                ]
                evmap = [[] for _ in range(8)]
                for ci, (c0, c1) in enumerate(SCR):
                    for b in range(8):
                        lo = max(c0, b * BLK)
                        hi = min(c1, (b + 1) * BLK, F)
                        if lo < hi:
                            evmap[b].append((ci, lo - b * BLK, hi - b * BLK,
                                             lo - c0))
                last_block = [min((c1 - 1) // BLK, 7) for (c0, c1) in SCR]

                def emit_stage2(ci):
                    c0, c1 = SCR[ci]
                    pit = c1 - c0
                    nc.sync.dma_start(scs[s][ci][:, :], pls[ci][:])
                    for ch, (u0, u1) in enumerate(GCH):
                        if CHUNK_SC[ch] != ci:
                            continue
                        ulen = u1 - u0
                        a = gath.tile([64, 1008], fp8, tag="a",
                                      name=f"a{s}_{ch}")
                        src = bass.AP(
                            scs[s][ci][:].tensor,
                            u0 - c0,
                            [[8 * pit + MS, 8], [pit + 1, 8], [1, ulen]],
                        )
                        if s == 0:
                            nc.gpsimd.dma_start(a[:, :ulen], src)
                            ats[(s, ch)] = a
                        else:
                            deferred_gathers.append((a, ulen, src))
                            ats[(s, ch)] = a

                for pi, (b0, nbk) in enumerate(PIECES):
                    lo = b0 * BLK
                    ln = min(nbk * BLK, F - lo)
                    sxp = {}
                    for k in range(2):
                        t = sxq.tile([128, 4 * BLK], bf16, tag="sxp",
                                     name=f"sxp{s}{k}{pi}")
                        nc.scalar.activation(
                            t[:, :ln], xst[(s, k, pi)][:, :ln], AF.Sqrt
                        )
                        sxp[k] = t
                    for j in range(nbk):
                        b = b0 + j
                        nb = min(BLK, F - b * BLK)
                        ps = psum.tile([64, BLK], f32, tag="ps",
                                       name=f"ps_{s}_{b}")
                        for k in range(2):
                            nc.tensor.matmul(
                                ps[:, :nb],
                                szw[:, k, :],
                                sxp[k][:, j * BLK : j * BLK + nb],
                                start=(k == 0),
                                stop=(k == 1),
                            )
                        for (ci, p_lo, p_hi, d_lo) in evmap[b]:
                            nc.vector.tensor_copy(
                                pls[ci][:, d_lo : d_lo + (p_hi - p_lo)],
                                ps[:, p_lo:p_hi],
                            )
                        for ci in range(len(SCR)):
                            if last_block[ci] == b:
                                emit_stage2(ci)

            # sample 1's gathers ride the (drained) Sync ring, issued
            # after all three of its dumps so no gather's completion wait
            # head-blocks a later dump's issue
            for (a, ulen, src_ap) in deferred_gathers:
                nc.sync.dma_start(a[:, :ulen], src_ap)

            # keep-warm: PE idles ~2us here waiting for the first gathers;
            # >3us idle re-throttles HAM to half clock for every tail
            # matmul.  Dummy matmuls on resident data bridge the gap
            # (results never read; WAR on the psum pool spaces them).
            for wi in range(6):
                pd = psum.tile([64, BLK], f32, tag="ps", name=f"warm{wi}")
                nc.tensor.matmul(
                    pd[:, :385],
                    szw[:, 0, :],
                    sxp[0][:, 0:385],
                    start=True,
                    stop=True,
                )

            # ---- stage-2 compute, strictly after both samples' stage-1
            # so engine program order matches data readiness
            for s in range(SPC):
                ps2 = psum2s[s]

                def mm2(ch):
                    u0, u1 = GCH[ch]
                    ulen = u1 - u0
                    row = 32 * ch
                    dst = ps2 if ch < 3 else psum3s[s]
                    a = ats[(s, ch)]
                    for m0 in range(0, ulen, BLK):
                        nb = min(BLK, ulen - m0)
                        nc.tensor.matmul(
                            dst[row : row + 1, m0 : m0 + nb],
                            ones[:],
                            a[:, m0 : m0 + nb],
                            start=True,
                            stop=True,
                            tile_position=(0, row),
                        )

                ob = obufs[s]
                obB = obufBs[s]
                # chunks 0-2 -> rows 0-47 ship without waiting on chunk
                # 3's dump/gather chain (it carries the last x block)
                for ch in range(3):
                    mm2(ch)
                nc.vector.tensor_copy(ob[0:33, :], ps2[0:33, :])
                nc.vector.tensor_copy(obB[64:65, :], ps2[64:65, :])
                osrc = bass.AP(ob[:].tensor, 0,
                               [[32 * 1024, 2], [MS, 16], [1, MO]])
                nc.sync.dma_start(out[s, 0, 0:32].unsqueeze(0), osrc)
                osrc = obB[64:65, 0 : 16 * MS].rearrange(
                    "p (i j) -> p i j", i=16
                )[:, :, 0:MO]
                nc.sync.dma_start(out[s, 0, 32:48].unsqueeze(0), osrc)
                mm2(3)
                ob3 = obuf3s[s]
                nc.scalar.copy(ob3[96:97, 0:BLK],
                               psum3s[s][96:97, 0:BLK])
                osrc = ob3[96:97, 0 : 8 * MS].rearrange(
                    "p (i j) -> p i j", i=8
                )[:, :, 0:MO]
                nc.sync.dma_start(out[s, 0, 48:MO].unsqueeze(0), osrc)

    nc.compile()
    return nc


def _get_nc():
    if "nc" not in _CACHE:
        _CACHE["nc"] = _build()
    return _CACHE["nc"]


def _run(z, x, weights, **runkw):
    z = np.ascontiguousarray(np.asarray(z), dtype=np.float32)
    x = np.ascontiguousarray(np.asarray(x), dtype=np.float32)
    w = np.ascontiguousarray(np.asarray(weights), dtype=np.float32).reshape(C)
    in_maps = []
    for i in range(NCORES):
        lo, hi = i * SPC, (i + 1) * SPC
        in_maps.append({"z": z[lo:hi], "x": x[lo:hi], "w": w})
    nc = _get_nc()
    try:
        res = run_bass_kernel_spmd(
            nc, in_maps, core_ids=list(range(NCORES)), **runkw
        )
    except Exception:
        res = run_bass_kernel_spmd(
            nc, in_maps, core_ids=list(range(NCORES)), **runkw
        )
    full = np.concatenate([res.results[i]["out"] for i in range(NCORES)], axis=0)
    return full, res


def kernel(z, x, weights):
    full, _ = _run(z, x, weights)
    return full


# revision 40
# speedup vs baseline: 1.1194x; 1.1194x over previous
"""Bhattacharyya coefficient kernel for Trainium2 (8 NeuronCores, SPMD).

out[n,0,i,j] = (1/k^2) * sum_{c,p,q} w[c] * sqrt(x[n,c,i+p,j+q] * z[n,c,p,q])

Data-parallel over batch, 2 samples per core.  Per sample:
  1. ACT: sx = sqrt(x) (bf16), szw = w/k^2 * sqrt(z) (bf16); sqrt(x*z)
     factorizes so the whole unfold collapses into a cross-correlation.
  2. TensorE: plane[t, y] = sum_c szw[c, t] * sx[c, y] for the 64 taps
     t = 8p+q and all 63*63 pixels y (K=256 as two accumulating
     128-chunks, 512-column PSUM blocks).
  3. DVE evicts PSUM into fp8-e4m3 plane pieces sized exactly like the
     DRAM scratch tensors (one dump DMA each, exact dependencies).
     fp8 halves scratch traffic; the tap-sum averages 64 independent
     quantization errors so absmax rel err stays ~1.1e-2 (< 2e-2 gate).
  4. Dump to DRAM scratch (Sync ring: FIFO behind the x stream, i.e.
     the transfers fill the ramp-down of the stream) and gather back
     tap-aligned with a flat-DRAM diagonal AP
       A[t, u] = plane[t, u + 63*(t>>3) + (t&7)]     (SWDGE ring),
     which turns the tap-sum into a pure partition reduction.
  5. Strictly after both samples' stage-1 (engines run in near-emission
     order; interleaving that mismatches readiness serializes the
     kernel): per chunk a K=64 ones-matmul, column-tiled so chunk ch
     accumulates on PSUM partition 32ch.  Chunk 3 (which carries the
     last x block) gets its own PSUM and SBUF tiles so neither its
     matmul nor its eviction waits on tile-granular hazards against the
     chunk 0-2 path, and its eviction runs on the idle ACT engine.
     Chunks 0+1 and chunk 2 evict into separate obuf tiles so each out
     DMA's tile-granular RAW covers only its own eviction - rows 0-31
     ship as soon as chunks 0/1 reduce.

The x stream owns the Sync HWDGE ring (12 piece loads issued
back-to-back after the tiny z/w loads) and runs at the ~358 GB/s HBM
limit for ~23us; everything else hides behind it except sample 1's
last-block tail.  Sample-1 gathers ride the drained Sync ring while sample-0
uses SWDGE, and keep-warm matmuls bridge the post-stage-1 PE gap so
HAM holds full clock into the tail.  Measured 47.5us (window-best) vs
the 55.5us baseline; run-to-run HW variance is +-3us.
"""

import numpy as np

import concourse.bacc as bacc
import concourse.bass as bass
import concourse.mybir as mybir
from concourse import tile
from concourse.bass_utils import run_bass_kernel_spmd

N, C, KS, MS = 16, 256, 8, 63
MO = MS - KS + 1            # 56
F = MS * MS                 # 3969
NCORES = 8
SPC = N // NCORES           # samples per core
BLK = 512
W = (MO - 1) * MS + MO      # 3521
SH = 448
AF = mybir.ActivationFunctionType
f32 = mybir.dt.float32
bf16 = mybir.dt.bfloat16
fp8 = mybir.dt.float8e4

PIECES = {0: [(0, 4), (4, 3), (7, 1)],
          1: [(0, 4), (4, 2), (6, 1), (7, 1)]}
GCH = [(0, 1008), (1008, 2016), (2016, 3024), (3024, W)]
SCR = [(0, 2016 + SH), (2016, 3024 + SH), (3024, F)]
CHUNK_SC = [0, 0, 1, 2]
OUT_ROWS = {1: (0, 32), 3: (32, MO)}

_CACHE = {}


def _build():
    nc = bacc.Bacc("TRN2", target_bir_lowering=False, debug=False)
    z_in = nc.declare_dram_parameter("z", [SPC, C, KS, KS], f32, isOutput=False)
    x_in = nc.declare_dram_parameter("x", [SPC, C, MS, MS], f32, isOutput=False)
    w_in = nc.declare_dram_parameter("w", [C], f32, isOutput=False)
    out = nc.declare_dram_parameter("out", [SPC, 1, MO, MO], f32, isOutput=True)

    scs = [
        [nc.dram_tensor(f"sc{ci}_{s}", [64, c1 - c0], fp8)
         for ci, (c0, c1) in enumerate(SCR)]
        for s in range(SPC)
    ]

    xflat = x_in.rearrange("s (k c) h w -> s k c (h w)", c=128)

    with tile.TileContext(nc) as tc:
        with (
            tc.tile_pool(name="xstage", bufs=12) as xstage,
            tc.tile_pool(name="sxq", bufs=5) as sxq,
            tc.tile_pool(name="zpool", bufs=2) as zpool,
            tc.tile_pool(name="plane", bufs=2) as plane,
            tc.tile_pool(name="gath", bufs=8) as gath,
            tc.tile_pool(name="opool", bufs=1) as opool,
            tc.tile_pool(name="psum", bufs=4, space="PSUM") as psum,
            tc.tile_pool(name="psum2", bufs=1, space="PSUM") as psum2,
            tc.tile_pool(name="psum3", bufs=2, space="PSUM") as psum3,
        ):
            xst = {}

            def load_piece(s, k, pi):
                b0, nbk = PIECES[s][pi]
                lo = b0 * BLK
                ln = min(nbk * BLK, F - lo)
                t = xstage.tile([128, 4 * BLK], f32, tag="xst",
                                name=f"xst{s}{k}{pi}")
                nc.sync.dma_start(t[:, :ln], xflat[s, k, :, lo : lo + ln])
                xst[(s, k, pi)] = t

            load_piece(0, 0, 0)
            wt = zpool.tile([128, 2], f32, name="wt")
            nc.sync.dma_start(wt[:], w_in.rearrange("(k c) -> c k", c=128))
            zts = []
            for s in range(SPC):
                zt = zpool.tile([128, 2, KS * KS], f32, tag="zt", name=f"zt{s}")
                nc.sync.dma_start(
                    zt[:], z_in[s].rearrange("(k c) p q -> c k (p q)", c=128)
                )
                zts.append(zt)
            for s in range(SPC):
                for pi in range(len(PIECES[s])):
                    for k in range(2):
                        if (s, k, pi) != (0, 0, 0):
                            load_piece(s, k, pi)

            ones = opool.tile([64, 1], fp8, name="ones")
            nc.gpsimd.memset(ones[:], 1.0)
            w64 = zpool.tile([128, 2], f32, name="w64")
            nc.vector.tensor_scalar_mul(w64[:], wt[:], 1.0 / (KS * KS))

            obufs, obufBs, obuf3s, psum2s, psum3s, ats = ({}, {}, {}, {},
                                                         {}, {})
            deferred_gathers = []
            for s in range(SPC):
                obuf = opool.tile([128, 1024], f32, tag=f"ob{s}",
                                  name=f"obuf{s}")
                obufs[s] = obuf
                obuf3s[s] = opool.tile([128, 512], f32, tag=f"ob3{s}",
                                       name=f"obuf3_{s}")
                obufBs[s] = opool.tile([128, 1024], f32, tag=f"obB{s}",
                                       name=f"obufB_{s}")
                psum2s[s] = psum2.tile([128, 2 * BLK], f32, tag="ps2",
                                       name=f"ps2_{s}")
                psum3s[s] = psum3.tile([128, BLK], f32, tag="ps3",
                                       name=f"ps3_{s}")
                zsq = zpool.tile([128, 2, KS * KS], f32, tag="zsq", name=f"zsq{s}")
                szw = zpool.tile([128, 2, KS * KS], bf16, tag="szw", name=f"szw{s}")
                for kk in range(2):
                    nc.scalar.activation(zsq[:, kk, :], zts[s][:, kk, :], AF.Sqrt)
                    nc.vector.tensor_scalar_mul(
                        szw[:, kk, :], zsq[:, kk, :], w64[:, kk : kk + 1]
                    )

                pls = [
                    plane.tile([64, c1 - c0], fp8, tag=f"pl{ci}",
                               name=f"pl{s}_{ci}")
                    for ci, (c0, c1) in enumerate(SCR)
                ]
                evmap = [[] for _ in range(8)]
                for ci, (c0, c1) in enumerate(SCR):
                    for b in range(8):
                        lo = max(c0, b * BLK)
                        hi = min(c1, (b + 1) * BLK, F)
                        if lo < hi:
                            evmap[b].append((ci, lo - b * BLK, hi - b * BLK,
                                             lo - c0))
                last_block = [min((c1 - 1) // BLK, 7) for (c0, c1) in SCR]

                def emit_stage2(ci):
                    c0, c1 = SCR[ci]
                    pit = c1 - c0
                    nc.sync.dma_start(scs[s][ci][:, :], pls[ci][:])
                    for ch, (u0, u1) in enumerate(GCH):
                        if CHUNK_SC[ch] != ci:
                            continue
                        ulen = u1 - u0
                        a = gath.tile([64, 1008], fp8, tag="a",
                                      name=f"a{s}_{ch}")
                        src = bass.AP(
                            scs[s][ci][:].tensor,
                            u0 - c0,
                            [[8 * pit + MS, 8], [pit + 1, 8], [1, ulen]],
                        )
                        if s == 0:
                            nc.gpsimd.dma_start(a[:, :ulen], src)
                            ats[(s, ch)] = a
                        else:
                            deferred_gathers.append((a, ulen, src))
                            ats[(s, ch)] = a

                for pi, (b0, nbk) in enumerate(PIECES[s]):
                    lo = b0 * BLK
                    ln = min(nbk * BLK, F - lo)
                    sxp = {}
                    for k in range(2):
                        t = sxq.tile([128, 4 * BLK], bf16, tag="sxp",
                                     name=f"sxp{s}{k}{pi}")
                        nc.scalar.activation(
                            t[:, :ln], xst[(s, k, pi)][:, :ln], AF.Sqrt
                        )
                        sxp[k] = t
                    for j in range(nbk):
                        b = b0 + j
                        nb = min(BLK, F - b * BLK)
                        ps = psum.tile([64, BLK], f32, tag="ps",
                                       name=f"ps_{s}_{b}")
                        for k in range(2):
                            nc.tensor.matmul(
                                ps[:, :nb],
                                szw[:, k, :],
                                sxp[k][:, j * BLK : j * BLK + nb],
                                start=(k == 0),
                                stop=(k == 1),
                            )
                        for (ci, p_lo, p_hi, d_lo) in evmap[b]:
                            nc.vector.tensor_copy(
                                pls[ci][:, d_lo : d_lo + (p_hi - p_lo)],
                                ps[:, p_lo:p_hi],
                            )
                        for ci in range(len(SCR)):
                            if last_block[ci] == b:
                                emit_stage2(ci)

            # sample 1's gathers ride the (drained) Sync ring, issued
            # after all three of its dumps so no gather's completion wait
            # head-blocks a later dump's issue
            for (a, ulen, src_ap) in deferred_gathers:
                nc.sync.dma_start(a[:, :ulen], src_ap)

            # keep-warm: PE idles ~2us here waiting for the first gathers;
            # >3us idle re-throttles HAM to half clock for every tail
            # matmul.  Dummy matmuls on resident data bridge the gap
            # (results never read; WAR on the psum pool spaces them).
            for wi in range(6):
                pd = psum.tile([64, BLK], f32, tag="ps", name=f"warm{wi}")
                nc.tensor.matmul(
                    pd[:, :385],
                    szw[:, 0, :],
                    sxp[0][:, 0:385],
                    start=True,
                    stop=True,
                )

            # ---- stage-2 compute, strictly after both samples' stage-1
            # so engine program order matches data readiness
            for s in range(SPC):
                ps2 = psum2s[s]

                def mm2(ch):
                    u0, u1 = GCH[ch]
                    ulen = u1 - u0
                    row = 32 * ch
                    dst = ps2 if ch < 3 else psum3s[s]
                    a = ats[(s, ch)]
                    for m0 in range(0, ulen, BLK):
                        nb = min(BLK, ulen - m0)
                        nc.tensor.matmul(
                            dst[row : row + 1, m0 : m0 + nb],
                            ones[:],
                            a[:, m0 : m0 + nb],
                            start=True,
                            stop=True,
                            tile_position=(0, row),
                        )

                ob = obufs[s]
                obB = obufBs[s]
                # chunks 0-2 -> rows 0-47 ship without waiting on chunk
                # 3's dump/gather chain (it carries the last x block)
                for ch in range(3):
                    mm2(ch)
                nc.vector.tensor_copy(ob[0:33, :], ps2[0:33, :])
                nc.vector.tensor_copy(obB[64:65, :], ps2[64:65, :])
                osrc = bass.AP(ob[:].tensor, 0,
                               [[32 * 1024, 2], [MS, 16], [1, MO]])
                nc.sync.dma_start(out[s, 0, 0:32].unsqueeze(0), osrc)
                osrc = obB[64:65, 0 : 16 * MS].rearrange(
                    "p (i j) -> p i j", i=16
                )[:, :, 0:MO]
                nc.sync.dma_start(out[s, 0, 32:48].unsqueeze(0), osrc)
                mm2(3)
                ob3 = obuf3s[s]
                nc.scalar.copy(ob3[96:97, 0:BLK],
                               psum3s[s][96:97, 0:BLK])
                osrc = ob3[96:97, 0 : 8 * MS].rearrange(
                    "p (i j) -> p i j", i=8
                )[:, :, 0:MO]
                nc.sync.dma_start(out[s, 0, 48:MO].unsqueeze(0), osrc)

    nc.compile()
    return nc


def _get_nc():
    if "nc" not in _CACHE:
        _CACHE["nc"] = _build()
    return _CACHE["nc"]


def _run(z, x, weights, **runkw):
    z = np.ascontiguousarray(np.asarray(z), dtype=np.float32)
    x = np.ascontiguousarray(np.asarray(x), dtype=np.float32)
    w = np.ascontiguousarray(np.asarray(weights), dtype=np.float32).reshape(C)
    in_maps = []
    for i in range(NCORES):
        lo, hi = i * SPC, (i + 1) * SPC
        in_maps.append({"z": z[lo:hi], "x": x[lo:hi], "w": w})
    nc = _get_nc()
    try:
        res = run_bass_kernel_spmd(
            nc, in_maps, core_ids=list(range(NCORES)), **runkw
        )
    except Exception:
        res = run_bass_kernel_spmd(
            nc, in_maps, core_ids=list(range(NCORES)), **runkw
        )
    full = np.concatenate([res.results[i]["out"] for i in range(NCORES)], axis=0)
    return full, res


def kernel(z, x, weights):
    full, _ = _run(z, x, weights)
    return full
